# revision 5
# baseline (speedup 1.0000x reference)
"""Trainium2 Bass kernel: out = e + e @ B @ A^T  (low-rank residual update).

e: [4, 4096, 4096] f32, A/B: [4096, 16] f32.  Tolerance is rel_err < 2e-2,
which admits bf16 transfers: all DRAM I/O is bf16 (host casts), halving HBM
traffic vs the f32 kernel (32 MiB/core vs 64 MiB -> ~94 us DMA floor vs 188).

Layout trick: the host ships each core's row-shard TRANSPOSED (eT [4096 d,
2048 r], contiguous).  On-device this kills the whole PE-transpose +
PSUM->SBUF copy pipeline the f32 kernel needed (the d-contraction wants d on
partitions):
  stage1: t[16, r] += B_k^T [128,16]^T @ eT_k [128, r]   (accum over 32 k)
  stage2: per 128-d' chunk c: yT_c [128, r] = at_c [16,128]^T @ t [16, r]
  add:    eT_c += yT_c  in place (DVE; optional ACT copy stage), store eT.
Host transposes the [4096, 2048] bf16 outputs back and casts f32.

Per-core engine budget @ st_rows=512 (TimelineSim-validated):
  DMA 16+16 MiB ~ 90 us (the bound), PE 2x65k cyc ~ 55 us, DVE/ACT adds+copies
  split to stay < 75 us each.
"""

import sys

sys.path.insert(0, "/opt/trn_rl_repo")

import numpy as np

import concourse.bass as bass
import concourse.mybir as mybir
import concourse.tile as tile


def _split_waits(nc, max_w=1):
    """The walrus in this container rejects instructions carrying more than
    ~2 sync-waits. Hoist extra waits onto same-engine NOPs placed directly
    before the offending instruction (engines execute their stream in
    order, so this is semantics-preserving)."""
    for f in nc.m.functions:
        for blk in f.blocks:
            insts = blk.instructions
            out = []
            changed = False
            for inst in insts:
                si = inst.sync_info
                if si is not None and si.on_wait and len(si.on_wait) > max_w:
                    waits = list(si.on_wait)
                    for j, w in enumerate(waits[max_w:]):
                        out.append(
                            mybir.InstNoOp(
                                name=f"{inst.name}-wsplit{j}",
                                sync_info=mybir.SyncInfo(on_wait=[w], on_update=[]),
                                bass_nofuse=True,
                                engine=inst.engine,
                            )
                        )
                    si.on_wait = waits[:max_w]
                    changed = True
                out.append(inst)
            if changed:
                blk.instructions = out


DIM = 4096
RANK = 16
N_CORES = 8
ROWS_TOTAL = 4 * 4096
ROWS_PER_CORE = ROWS_TOTAL // N_CORES  # 2048

BF16 = mybir.dt.bfloat16
F32 = mybir.dt.float32


def build_nc(rows_per_core=ROWS_PER_CORE, st_rows=512, add_mode="split", grp=2,
             e_bufs=4, y_bufs=2, reps=1, split_waits=True, store_eng="scalar",
             store_halves=True, t_eng="vector", mix=0):
    """add_mode: 'direct' = DVE tensor_add with f32 PSUM operand (1x mode);
    'split' = ACT copies yT PSUM->SBUF bf16, DVE adds bf16+bf16 (2x mode);
    'mix'   = alternate: groups g with (g % 4) < mix use split, rest direct
              (balances the 1x-PSUM-read cost across ACT and DVE).
    grp: d'-chunks per PSUM tile (bigger FD amortizes DVE/ACT fixed cost)."""
    kc = DIM // 128  # 32 k-tiles / d'-chunks
    assert rows_per_core % st_rows == 0 and kc % grp == 0
    n_st = rows_per_core // st_rows
    n_grp = kc // grp

    nc = bass.Bass("TRN2", target_bir_lowering=False, debug=False)
    # host ships eT pre-tiled [n_st, 128 p, kc, st_rows] (d = k*128+p,
    # row = st*st_rows+r) so every DMA is [128, kc*st_rows] with per-partition
    # contiguous 2*kc*st_rows bytes -> 128 maximal descriptors per transfer
    et_in = nc.dram_tensor(
        "et_in", [n_st * 128, kc * st_rows], BF16, kind="ExternalInput"
    )
    b_in = nc.dram_tensor("b_in", [DIM, RANK], BF16, kind="ExternalInput")
    at_in = nc.dram_tensor("at_in", [RANK, DIM], BF16, kind="ExternalInput")
    out_d = nc.dram_tensor(
        "out_d", [n_st * 128, kc * st_rows], BF16, kind="ExternalOutput"
    )

    with tile.TileContext(nc) as tc:
        with (
            tc.tile_pool(name="const", bufs=1) as cpool,
            tc.tile_pool(name="epool", bufs=e_bufs) as epool,
            tc.tile_pool(name="tpool", bufs=2) as tpool,
            tc.tile_pool(name="ypool", bufs=max(2 * grp, 4)) as ypool,
            tc.tile_pool(name="pst", bufs=2, space="PSUM") as pst,
            tc.tile_pool(name="psy", bufs=y_bufs, space="PSUM") as psy,
        ):
            b_sb = cpool.tile([128, kc * RANK], BF16, name="b_sb")
            nc.sync.dma_start(
                out=b_sb.rearrange("p (k j) -> p k j", j=RANK),
                in_=b_in.ap().rearrange("(k p) j -> p k j", p=128),
            )
            at_sb = cpool.tile([RANK, DIM], BF16, name="at_sb")
            nc.sync.dma_start(out=at_sb, in_=at_in.ap()[:, :])

            et_ap = et_in.ap()
            o_ap = out_d.ap()

            ctx = {}

            def emit_load(st):
                p0 = (st % n_st) * 128
                et = epool.tile([128, kc * st_rows], BF16, name="et")
                nc.sync.dma_start(out=et, in_=et_ap[p0 : p0 + 128, :])
                ctx[st] = {"et": et}

            def emit_s1(st):
                c = ctx[st]
                t_ps = pst.tile([RANK, st_rows], F32, name="t_ps")
                et = c["et"]
                for k in range(kc):
                    nc.tensor.matmul(
                        t_ps,
                        b_sb[:, k * RANK : (k + 1) * RANK],
                        et[:, k * st_rows : (k + 1) * st_rows],
                        start=(k == 0),
                        stop=(k == kc - 1),
                    )
                t_sb = tpool.tile([RANK, st_rows], BF16, name="t_sb")
                if t_eng == "vector":
                    nc.vector.tensor_copy(out=t_sb, in_=t_ps)
                else:
                    nc.scalar.copy(t_sb, t_ps)
                c["t_sb"] = t_sb

            def emit_s2_grp(st, g):
                c = ctx[st]
                et, t_sb = c["et"], c["t_sb"]
                yp = psy.tile([128, grp * st_rows], F32, name="yp")
                for j in range(grp):
                    ch = g * grp + j
                    nc.tensor.matmul(
                        yp[:, j * st_rows : (j + 1) * st_rows],
                        at_sb[:, ch * 128 : (ch + 1) * 128],
                        t_sb,
                        start=True,
                        stop=True,
                    )
                sl = slice(g * grp * st_rows, (g + 1) * grp * st_rows)
                use_split = add_mode == "split" or (
                    add_mode == "mix" and (g % 4) < mix
                )
                if not use_split:
                    nc.vector.tensor_add(out=et[:, sl], in0=et[:, sl], in1=yp)
                else:
                    ysb = ypool.tile([128, grp * st_rows], BF16, name="ysb")
                    nc.scalar.copy(ysb, yp)
                    nc.vector.tensor_add(out=et[:, sl], in0=et[:, sl], in1=ysb)
                # store as soon as each half of the supertile's adds are done
                # (halves the WAR-release latency on the et slot)
                p0 = (st % n_st) * 128
                if store_halves:
                    if (g + 1) % (n_grp // 2) == 0:
                        h = (g + 1) // (n_grp // 2) - 1
                        cs = slice(h * kc * st_rows // 2, (h + 1) * kc * st_rows // 2)
                        getattr(nc, store_eng).dma_start(
                            out=o_ap[p0 : p0 + 128, cs], in_=et[:, cs]
                        )
                elif g == n_grp - 1:
                    getattr(nc, store_eng).dma_start(
                        out=o_ap[p0 : p0 + 128, :], in_=et[:, :]
                    )
                if g == n_grp - 1:
                    del ctx[st]

            total_st = n_st * reps  # reps>1: timing-only in-NEFF repeat
            for st in range(total_st):
                emit_load(st)
                emit_s1(st)
                for g in range(n_grp):
                    emit_s2_grp(st, g)

    if split_waits:
        _split_waits(nc)
    return nc


_NC_CACHE = {}

ST_ROWS = 256
BUILD_KW = dict(
    st_rows=ST_ROWS, e_bufs=10, add_mode="mix", mix=3, t_eng="scalar",
    store_eng="gpsimd", y_bufs=3,
)


def _get_nc(rows_per_core=ROWS_PER_CORE):
    key = rows_per_core
    if key not in _NC_CACHE:
        _NC_CACHE[key] = build_nc(rows_per_core, **BUILD_KW)
    return _NC_CACHE[key]


def _pack(e_shard_f32, st_rows, bf16):
    # [rows, DIM] f32 -> [n_st*128, kc*st_rows] bf16 tiled as [st][p][k][r]
    rows = e_shard_f32.shape[0]
    n_st = rows // st_rows
    a = e_shard_f32.reshape(n_st, st_rows, DIM // 128, 128)  # [st, r, k, p]
    a = a.transpose(0, 3, 2, 1).astype(bf16)  # [st, p, k, r]
    return np.ascontiguousarray(a.reshape(n_st * 128, -1))


def _unpack(o_tiled, st_rows):
    # [n_st*128, kc*st_rows] bf16 -> [rows, DIM] f32
    n_st = o_tiled.shape[0] // 128
    a = o_tiled.reshape(n_st, 128, DIM // 128, st_rows).astype(np.float32)
    return a.transpose(0, 3, 2, 1).reshape(n_st * st_rows, DIM)


def kernel(e, A, B):
    from concourse.bass_utils import run_bass_kernel_spmd
    import ml_dtypes

    bf16 = ml_dtypes.bfloat16
    e = np.asarray(e, dtype=np.float32)
    A = np.asarray(A, dtype=np.float32)
    B = np.asarray(B, dtype=np.float32)
    batch, seq, dim = e.shape
    rows = batch * seq
    e2 = e.reshape(rows, dim)
    rpc = rows // N_CORES

    b_bf = B.astype(bf16)
    at_bf = np.ascontiguousarray(A.T).astype(bf16)
    in_maps = [
        {
            "et_in": _pack(e2[i * rpc : (i + 1) * rpc], ST_ROWS, bf16),
            "b_in": b_bf,
            "at_in": at_bf,
        }
        for i in range(N_CORES)
    ]
    nc = _get_nc(rpc)
    res = run_bass_kernel_spmd(nc, in_maps, core_ids=list(range(N_CORES)))
    out = np.empty((rows, dim), dtype=np.float32)
    for i in range(N_CORES):
        out[i * rpc : (i + 1) * rpc] = _unpack(res.results[i]["out_d"], ST_ROWS)
    return out.reshape(batch, seq, dim)
